# revision 24
# baseline (speedup 1.0000x reference)
"""VQ codebook kernel (nn_KW_CascadedBranch) for 8 Trainium2 NeuronCores.

Reference computation:
    kw   = audio_feat @ proj_w + proj_b                  [B,N,512]
    cos  = normalize(kw) @ normalize(token_embedding).T  [B,N,V]
    p    = softmax(cos / 0.1)
    out  = p @ token_embedding                           [B,N,512]

Strategy: tensor-parallel over the vocab dim V=49408. Each core owns a
6176-row shard (padded to 6272 = 49*128), keeps the transposed shard
resident in SBUF, and computes partial (p @ emb) and partial softmax
denominators for ALL B*N=2048 keyword slots. Softmax needs no max
subtraction: logits = 10*cos are in [-10,10], so exp() is safe in fp32,
and partial sums are exact to combine: out = (sum_c pe_c) / (sum_c d_c).
Host combines the 8 partials (a [512,2048] add) and divides.

Per core the two big GEMMs run on the PE in float32r at 1 cycle/row:
  GEMM1 scores^T[v,m] = emb_t(lhsT) @ kw_n^T(rhs), accumulated over e
  GEMM2 out^T[e,m]   += emb(lhsT)   @ p^T(rhs),    accumulated over v
The exp fuses vocab-side normalization (scale = 10/||emb_v||, an AP) and
the shard-padding mask (bias = -1e30 on pad rows) into one ACT pass.
"""

import numpy as np

import concourse.bass as bass
import concourse.mybir as mybir
from concourse import tile
from concourse.bass_utils import run_bass_kernel_spmd

F32 = mybir.dt.float32
F32R = mybir.dt.float32r
AF = mybir.ActivationFunctionType
OP = mybir.AluOpType

N_CORES = 8
B, N, D, E, V = 256, 8, 768, 512, 49408
M = B * N                      # 2048 keyword slots
VS = V // N_CORES              # 6176 real vocab rows per core
VT = 49                        # v-tiles of 128 per core (6272 rows, 96 pad)
VP = VT * 128
MC = 512                       # m-chunk (columns per PSUM accumulator)
NMC = M // MC                  # 4
MT = M // 128                  # 16 m-tiles in the projection prologue
DT = D // 128                  # 6 d-chunks
EC = E // 128                  # 4 e-chunks
INV_TEMP = 10.0                # 1/T
NEG_BIG = -1.0e30
SC_BUFS = 2                    # scores PSUM double-buffer depth
EN_BUFS = 4                    # emb-natural stream prefetch depth
P_BUFS = 4                     # p tile depth


def r32(ap):
    return ap.bitcast(F32R)


def _split_multiwait_ctrl(nc, max_waits: int = 1) -> int:
    """This container's walrus rejects instructions carrying more than one
    semaphore wait (CTRL and S3_LW encodings alike). Hoist overflow waits
    onto same-engine NoOps inserted immediately before the offender."""
    n_split = 0
    for fn in nc.m.functions:
        for bb in fn.blocks:
            rebuilt, changed = [], False
            for ins in bb.instructions:
                si = ins.sync_info
                if (
                    si is not None
                    and si.on_wait
                    and len(si.on_wait) > max_waits
                ):
                    waits = list(si.on_wait)
                    head, tail = waits[:-max_waits], waits[-max_waits:]
                    for i in range(0, len(head), max_waits):
                        nop = mybir.InstNoOp(name=f"{ins.name}-ws{i}", ins=[], outs=[])
                        nop.engine = ins.engine
                        nop.sync_info = mybir.SyncInfo(
                            on_wait=head[i:i + max_waits], on_update=[]
                        )
                        rebuilt.append(nop)
                    ins.sync_info = mybir.SyncInfo(
                        on_wait=tail, on_update=list(si.on_update or [])
                    )
                    changed = True
                    n_split += 1
                rebuilt.append(ins)
            if changed:
                bb.instructions = rebuilt
    return n_split


def build_program():
    nc = bass.Bass(target_bir_lowering=False)

    audio_t = nc.dram_tensor("audio_t", [D, M], F32R, kind="ExternalInput")
    proj_w = nc.dram_tensor("proj_w", [D, E], F32R, kind="ExternalInput")
    proj_b = nc.dram_tensor("proj_b", [1, E], F32, kind="ExternalInput")
    emb = nc.dram_tensor("emb", [VP, E], F32R, kind="ExternalInput")
    emb_t = nc.dram_tensor("emb_t", [EC, 128, VP], F32R, kind="ExternalInput")
    mask_b = nc.dram_tensor("mask_b", [128, VT], F32, kind="ExternalInput")
    ident = nc.dram_tensor("ident", [128, 128], F32, kind="ExternalInput")

    out_pe = nc.dram_tensor("out_pe", [E, M], F32, kind="ExternalOutput")
    out_d = nc.dram_tensor("out_d", [1, M], F32, kind="ExternalOutput")

    with tile.TileContext(nc) as tc:
        with (
            tc.tile_pool(name="resident", bufs=1) as res,
            tc.tile_pool(name="small", bufs=1) as small,
        ):
            # ---- resident SBUF tensors ----
            et_sb = [res.tile([128, VP], F32R, tag=f"et{j}", name=f"et{j}") for j in range(EC)]
            PIECE = VP // 4
            for j in range(EC):
                for pc in range(4):
                    sl = slice(pc * PIECE, (pc + 1) * PIECE)
                    nc.sync.dma_start(et_sb[j][:, sl], emb_t[j][:, sl])
            kwnT = [
                [
                    res.tile([128, MC], F32R, tag=f"kwnT{j}_{c}", name=f"kwnT{j}_{c}")
                    for c in range(NMC)
                ]
                for j in range(EC)
            ]
            mask_sb = small.tile([128, VT], F32, tag="mask")
            nc.sync.dma_start(mask_sb[:], mask_b[:])
            id_sb = small.tile([128, 128], F32, tag="ident")
            nc.sync.dma_start(id_sb[:], ident[:])
            ones_col = small.tile([128, 1], F32, tag="ones_col")
            nc.vector.memset(ones_col[:], 1.0)
            ones_row = small.tile([1, 128], F32, tag="ones_row")
            nc.vector.memset(ones_row[:], 1.0)
            pb_sb = small.tile([1, E], F32, tag="pb")
            nc.sync.dma_start(pb_sb[:], proj_b[:])
            scale_e = small.tile([128, VT], F32, tag="scale_e")

            # ---- prologue ----
            with (
                tc.tile_pool(name="pro", bufs=2) as pro,
                tc.tile_pool(name="prok", bufs=4) as prok,
                tc.tile_pool(name="pro1", bufs=1) as pro1,
                tc.tile_pool(name="pro_ps", bufs=2, space="PSUM") as pro_ps,
                tc.tile_pool(name="pro_ps2", bufs=2, space="PSUM") as pro_ps2,
            ):
                # vocab-shard row norms from the resident transposed copy:
                # square 896-column pieces on ACT, then reduce over e with
                # squared-slice-as-stationary matmuls -> normsq lands [v, 1].
                ensq = pro1.tile([128, VT], F32, tag="ensq")
                PW = VP // 7  # 896 columns = 7 v-tiles per piece
                for p in range(7):
                    psl = slice(p * PW, (p + 1) * PW)
                    sqs = [
                        pro1.tile([128, PW], F32, tag=f"sqs{j}", name=f"sqs{j}")
                        for j in range(EC)
                    ]
                    for j in range(EC):
                        nc.scalar.activation(
                            sqs[j][:], et_sb[j][:, psl].bitcast(F32), AF.Square
                        )
                    for t in range(7):
                        k = p * 7 + t
                        nq = pro_ps2.tile([128, 1], F32, tag="nq")
                        for j in range(EC):
                            nc.tensor.matmul(
                                nq[:], sqs[j][:, t * 128:(t + 1) * 128], ones_col[:],
                                start=(j == 0), stop=(j == EC - 1),
                            )
                        nc.vector.tensor_copy(ensq[:, k:k + 1], nq[:])
                # scale_e = 10 * rsqrt(ensq): sqrt -> recip -> one Newton step
                # (+1e-24 keeps the all-zero pad rows finite through the chain)
                nc.vector.tensor_scalar_add(ensq[:], ensq[:], 1e-24)
                s_e = pro1.tile([128, VT], F32, tag="s_e")
                nc.scalar.activation(s_e[:], ensq[:], AF.Sqrt)
                r0 = pro1.tile([128, VT], F32, tag="r0_e")
                nc.vector.reciprocal(r0[:], s_e[:])
                t0 = pro1.tile([128, VT], F32, tag="t0_e")
                nc.vector.tensor_mul(t0[:], r0[:], r0[:])
                nc.vector.tensor_mul(t0[:], t0[:], ensq[:])
                nc.vector.tensor_scalar(t0[:], t0[:], -0.5, 1.5, OP.mult, OP.add)
                nc.vector.tensor_mul(t0[:], t0[:], r0[:])
                nc.vector.tensor_scalar_mul(scale_e[:], t0[:], INV_TEMP)

                # proj_b broadcast to all 128 partitions (rank-1 matmul)
                bb_ps = pro_ps2.tile([128, E], F32, tag="bb_ps")
                nc.tensor.matmul(bb_ps[:], ones_row[:], pb_sb[:])
                bcast_b = pro1.tile([128, E], F32, tag="bcast_b")
                nc.vector.tensor_copy(bcast_b[:], bb_ps[:])

                # keyword projection + row normalization + transpose
                pw = [pro1.tile([128, E], F32R, tag=f"pw{d}", name=f"pw{d}") for d in range(DT)]
                for d in range(DT):
                    nc.sync.dma_start(pw[d][:], proj_w[d * 128:(d + 1) * 128, :])
                for i in range(MT):
                    at = [prok.tile([128, 128], F32R, tag=f"at{d}", name=f"at{d}") for d in range(DT)]
                    for d in range(DT):
                        nc.sync.dma_start(
                            at[d][:],
                            audio_t[d * 128:(d + 1) * 128, i * 128:(i + 1) * 128],
                        )
                    kw_ps = pro_ps.tile([128, E], F32, tag="kw_ps")
                    for d in range(DT):
                        nc.tensor.matmul(
                            kw_ps[:], at[d][:], pw[d][:],
                            start=(d == 0), stop=(d == DT - 1),
                        )
                    kw_sb = prok.tile([128, E], F32, tag="kw_sb")
                    nc.vector.tensor_add(kw_sb[:], kw_ps[:], bcast_b[:])
                    # row norm -> rsqrt (Newton-refined)
                    sq = prok.tile([128, E], F32, tag="sq_kw")
                    nsq = prok.tile([128, 1], F32, tag="nsq_kw")
                    nc.scalar.activation(
                        sq[:], kw_sb[:], AF.Square, accum_out=nsq[:],
                    )
                    sk = prok.tile([128, 1], F32, tag="sk")
                    nc.scalar.activation(sk[:], nsq[:], AF.Sqrt)
                    rk = prok.tile([128, 1], F32, tag="rk")
                    nc.vector.reciprocal(rk[:], sk[:])
                    tk = prok.tile([128, 1], F32, tag="tk")
                    nc.vector.tensor_mul(tk[:], rk[:], rk[:])
                    nc.vector.tensor_mul(tk[:], tk[:], nsq[:])
                    nc.vector.tensor_scalar(tk[:], tk[:], -0.5, 1.5, OP.mult, OP.add)
                    nc.vector.tensor_mul(tk[:], tk[:], rk[:])
                    kwn = prok.tile([128, E], F32, tag="kwn")
                    nc.vector.tensor_scalar_mul(kwn[:], kw_sb[:], tk[:])
                    for j in range(EC):
                        tp = pro_ps2.tile([128, 128], F32, tag="tp")
                        nc.tensor.transpose(
                            tp[:], kwn[:, j * 128:(j + 1) * 128], id_sb[:]
                        )
                        nc.any.tensor_copy(
                            kwnT[j][i // 4][:, (i % 4) * 128:(i % 4 + 1) * 128],
                            tp[:],
                        )

            # ---- main loop ----
            with (
                tc.tile_pool(name="sc_ps", bufs=SC_BUFS, space="PSUM") as sc_ps,
                tc.tile_pool(name="acc_ps", bufs=5, space="PSUM") as acc_ps,
                tc.tile_pool(name="d_ps", bufs=1, space="PSUM") as d_ps,
                tc.tile_pool(name="mn", bufs=P_BUFS) as mn,
                tc.tile_pool(name="mn1", bufs=2) as mn1,
                tc.tile_pool(name="enp", bufs=EN_BUFS) as enp,
            ):
                for mc in range(NMC):
                    m0 = mc * MC
                    kwacc = [
                        acc_ps.tile([128, MC], F32, tag="kwacc", name=f"kwacc{j}")
                        for j in range(EC)
                    ]
                    dacc = mn1.tile([128, MC], F32, tag="dacc")
                    for k in range(VT):
                        scores = sc_ps.tile([128, MC], F32, tag="scores")
                        for j in range(EC):
                            nc.tensor.matmul(
                                scores[:],
                                et_sb[j][:, k * 128:(k + 1) * 128],
                                kwnT[j][mc][:],
                                start=(j == 0), stop=(j == EC - 1),
                            )
                        p_sb = mn.tile([128, MC], F32R, tag="p")
                        nc.scalar.activation(
                            p_sb[:], scores[:], AF.Exp,
                            bias=mask_sb[:, k:k + 1],
                            scale=scale_e[:, k:k + 1],
                        )
                        if k == 0:
                            nc.vector.tensor_copy(dacc[:], p_sb[:].bitcast(F32))
                        else:
                            nc.vector.tensor_add(dacc[:], dacc[:], p_sb[:].bitcast(F32))
                        en = enp.tile([128, E], F32R, tag="en")
                        nc.sync.dma_start(en[:], emb[k * 128:(k + 1) * 128, :])
                        for j in range(EC):
                            nc.tensor.matmul(
                                kwacc[j][:],
                                en[:, j * 128:(j + 1) * 128],
                                p_sb[:],
                                start=(k == 0), stop=(k == VT - 1),
                            )
                    dred = d_ps.tile([1, MC], F32, tag="dred")
                    nc.tensor.matmul(dred[:], ones_col[:], dacc[:])
                    dsb = mn.tile([1, MC], F32, tag="dsb")
                    nc.scalar.copy(dsb[:], dred[:])
                    nc.sync.dma_start(out_d[:, m0:m0 + MC], dsb[:])
                    for j in range(EC):
                        osb = mn.tile([128, MC], F32, tag="osb")
                        nc.any.tensor_copy(osb[:], kwacc[j][:])
                        nc.sync.dma_start(
                            out_pe[j * 128:(j + 1) * 128, m0:m0 + MC], osb[:]
                        )
    return nc


_CACHED = {}


def _get_program():
    if "nc" not in _CACHED:
        nc = build_program()
        _split_multiwait_ctrl(nc)
        _CACHED["nc"] = nc
    return _CACHED["nc"]


def _prep_in_maps(audio_feat, proj_w, proj_b, token_embedding):
    audio = np.ascontiguousarray(np.asarray(audio_feat, np.float32))
    pw = np.ascontiguousarray(np.asarray(proj_w, np.float32))
    pb = np.ascontiguousarray(np.asarray(proj_b, np.float32)).reshape(1, E)
    emb = np.ascontiguousarray(np.asarray(token_embedding, np.float32))

    audio_t = np.ascontiguousarray(audio.reshape(M, D).T)
    mask = np.zeros((128, VT), np.float32)
    nreal_last = VS - (VT - 1) * 128          # 32 real rows in the last v-tile
    mask[nreal_last:, VT - 1] = NEG_BIG
    ident = np.eye(128, dtype=np.float32)

    in_maps = []
    for c in range(N_CORES):
        shard = np.zeros((VP, E), np.float32)
        shard[:VS] = emb[c * VS:(c + 1) * VS]
        shard_t = np.ascontiguousarray(shard.T).reshape(EC, 128, VP)
        in_maps.append({
            "audio_t": audio_t,
            "proj_w": pw,
            "proj_b": pb,
            "emb": shard,
            "emb_t": shard_t,
            "mask_b": mask,
            "ident": ident,
        })
    return in_maps


def kernel(audio_feat, proj_w, proj_b, token_embedding, _trace=False):
    nc = _get_program()
    in_maps = _prep_in_maps(audio_feat, proj_w, proj_b, token_embedding)
    res = run_bass_kernel_spmd(
        nc, in_maps, core_ids=list(range(N_CORES)), trace=_trace
    )
    pe = np.zeros((E, M), np.float64)
    dn = np.zeros((1, M), np.float64)
    for c in range(N_CORES):
        pe += res.results[c]["out_pe"]
        dn += res.results[c]["out_d"]
    out = (pe / dn).T.reshape(B, N, E).astype(np.float32)
    if _trace:
        return out, res
    return out
